# revision 39
# baseline (speedup 1.0000x reference)
"""Causal attention (B=8, S=2048, D=768, single head) on 8 trn2 NeuronCores.

Sharding: data-parallel over batch — core b computes batch element b.

All matmul operands are bf16 (f32 accumulation in PSUM); ~5e-3 rel err,
well inside the 2e-2 gate.

Algebraic trick: scores = Q K^T = x (Wq Wk^T) x^T. M = Wq @ Wk^T is
computed at startup from the weights alone, so EVERY chunk needs just
one projection B = (x M)^T and contracts scores^T = XT.T @ B — the
separate Q and K projections disappear entirely.

x^T production: chunk 0 and chunk 1 are cast-loaded to bf16 by the
SWDGE (gpsimd) ring (which also cast-loads wv straight into its bf16
tile) and PE-transposed; chunks 2-3 stage f32 on the HWDGE rings
mid-kernel and PE-transpose as f32r. Transposes are grouped per
o-block so each PSUM->SBUF copy lands contiguously, from a dedicated
2-bank PSUM pool so their (slow) DVE copies never backpressure the
matmul pools.

Startup: wq/wk f32 pieces stream on the two HWDGE rings (wq piece 0
split in half so the first PE transpose starts ~1.5us in) and are
f32r-PE-transposed as they arrive, interleaved with x(0) transposes
and HAM-warming dummy matmuls; M's 256-wide matmul groups run as wk
piece pairs complete, with B(0) eb-pieces chasing their M column
group (eb needs only group eb//2) to fill the wk-DMA wait.

Per-core pipeline (fused over 512-wide s-chunks):
  1. B = M-block.T @ xT per chunk
  2. scores^T tile [k_part, q]: contract d via XT, B
  3. exp (scale fused) on ACT -> PT bf16; triangular mask on corner block
  4. V projection for the chunk (PSUM copies alternate ACT/DVE);
     next chunk's x transposes run mid-V-phase
  5. out[q, e] (+denominator via ones cols of V) = PT_blk.T @ V_blk;
     the last block splits into two 256-wide PSUM groups so its
     mul+store tail hides under matmuls
  6. normalize via reciprocal (DVE) + scaled copy (ACT), stores on sync
"""

import numpy as np

import concourse.bass as bass
import concourse.mybir as mybir
from concourse import bacc
from concourse.tile import TileContext
from concourse.bass_utils import run_bass_kernel_spmd
from concourse.masks import make_identity

B, S, D = 8, 2048, 768
P = 128
ND = D // P            # 6 feature blocks
NB = S // P            # 16 seq blocks
CH = 512               # s-chunk width
NCH = S // CH          # 4 chunks
QPC = CH // P          # 4 q/s-blocks per chunk
SCALE = 1.0 / float(np.sqrt(D))
F32 = mybir.dt.float32
F32R = mybir.dt.float32r
BF16 = mybir.dt.bfloat16
EXP = mybir.ActivationFunctionType.Exp


def _build_nc():
    nc = bacc.Bacc(None, target_bir_lowering=False)
    xb = nc.dram_tensor("xb", [S, D], F32, kind="ExternalInput")
    wq_d = nc.dram_tensor("wq", [D, D], F32, kind="ExternalInput")
    wk_d = nc.dram_tensor("wk", [D, D], F32, kind="ExternalInput")
    wv_d = nc.dram_tensor("wv", [D, D], F32, kind="ExternalInput")
    out_d = nc.dram_tensor("out", [S, D], F32, kind="ExternalOutput")


    # [d, e] weight views as [d_in(128), d_block(6), e(768)]
    wq_r = wq_d[:, :].rearrange("(o p) e -> p o e", p=P)
    wk_r = wk_d[:, :].rearrange("(o p) e -> p o e", p=P)
    wv_r = wv_d[:, :].rearrange("(o p) e -> p o e", p=P)

    with TileContext(nc) as tc:
        with (
            tc.tile_pool(name="const", bufs=1) as constp,
            tc.tile_pool(name="persist", bufs=1) as persist,
            tc.tile_pool(name="whstage", bufs=2) as whstage,
            tc.tile_pool(name="wstage", bufs=6) as wstage,
            tc.tile_pool(name="xfload", bufs=4) as xfload,
            tc.tile_pool(name="xload", bufs=2) as xload,
            tc.tile_pool(name="xrload", bufs=8) as xrload,
            tc.tile_pool(name="qt", bufs=2) as qtp,
            tc.tile_pool(name="outp", bufs=3) as outp,
            tc.tile_pool(name="rc", bufs=4) as rcp,
            tc.tile_pool(name="psW", bufs=3, space="PSUM") as psW,
            tc.tile_pool(name="psO", bufs=3, space="PSUM") as psO,
            tc.tile_pool(name="psT", bufs=2, space="PSUM") as psT,
        ):
            ident = constp.tile([P, P], F32)
            make_identity(nc, ident)
            ident_r = constp.tile([P, P], F32R)
            nc.vector.tensor_copy(ident_r, ident)
            ident_b = constp.tile([P, P], BF16)
            nc.vector.tensor_copy(ident_b, ident)
            # tri[p, j] = 1.0 if p <= j else 0.0 (keep k <= q on the
            # diagonal 128x128 corner of each score block)
            tri = constp.tile([P, P], BF16)
            nc.gpsimd.memset(tri, 1.0)
            nc.gpsimd.affine_select(
                out=tri,
                in_=tri,
                compare_op=mybir.AluOpType.is_ge,
                fill=0.0,
                base=0,
                pattern=[[1, P]],
                channel_multiplier=-1,
            )

            WqT = persist.tile([P, ND, D], BF16)     # Wq^T: [e_in, eo, d]
            WkT = persist.tile([P, ND, D], BF16)     # Wk^T: [e_in, eo, d]
            Mt = persist.tile([P, ND, D], BF16)      # M:    [d'_in, o', d]
            WV = persist.tile([P, ND, D], BF16)
            XT = persist.tile([P, ND, S], BF16)      # x^T, all chunks
            V = persist.tile([P, NB, D + 2], BF16)   # [s_in, sb, e]; cols D..D+1 = 1.0
            PT = persist.tile([P, NB, CH], BF16)     # exp(scores^T) blocks of chunk
            ones_col = constp.tile([P, NB, 2], BF16)
            nc.vector.memset(ones_col, 1.0)
            nc.vector.tensor_copy(V[:, :, D : D + 2], ones_col)

            # ---- startup DMAs. wq/wk f32 pieces split across the two
            # HWDGE rings (sync/scalar); x chunk 0 cast-loads to bf16 on
            # the gpsimd SWDGE ring, followed by wv and chunk 1.
            # wq piece 0 splits into two half-pieces so the first PE
            # transpose can start ~1.5us earlier
            wq0_half = []
            for h in range(2):
                s = whstage.tile([P, 1, D // 2], F32R, tag="wh", name=f"wq0h{h}")
                eng = nc.sync if h == 0 else nc.scalar
                eng.dma_start(
                    s, wq_r[:, 0:1, h * (D // 2) : (h + 1) * (D // 2)].bitcast(F32R)
                )
                wq0_half.append(s)
            wq_stage = [None]
            wk_stage = []
            for o in range(1, ND):
                s = wstage.tile([P, 1, D], F32R, tag="ws", name=f"wq{o}")
                eng = nc.sync if o % 2 == 0 else nc.scalar
                eng.dma_start(s, wq_r[:, o : o + 1, :].bitcast(F32R))
                wq_stage.append(s)
            for o in range(ND):
                s = wstage.tile([P, 1, D], F32R, tag="ws", name=f"wk{o}")
                eng = nc.sync if o % 2 == 0 else nc.scalar
                eng.dma_start(s, wk_r[:, o : o + 1, :].bitcast(F32R))
                wk_stage.append(s)
            xf_tiles = []
            for sb in range(QPC):
                xf = xfload.tile([P, D], BF16, tag="xf", name=f"xf{sb}")
                nc.gpsimd.dma_start(xf, xb[sb * P : (sb + 1) * P, :])
                xf_tiles.append(xf)
            # wv cast-loads straight into WV on the SWDGE ring behind
            # x(0); chunk 1's x follows as a bf16 cast-load. Chunks 2-3
            # stage f32 on the HWDGE rings mid-kernel (keeping the rings
            # clear of x traffic during the startup weight burst).
            nc.gpsimd.dma_start(WV, wv_r)
            xc_tiles = {}
            xc = xload.tile([P, QPC, D], BF16, tag="xc", name="xc1")
            nc.gpsimd.dma_start(
                xc, xb[CH : 2 * CH, :].rearrange("(s p) d -> p s d", p=P)
            )
            xc_tiles[1] = xc
            xr_tiles = {}

            def emit_xr_load(c, sb, eng):
                xr = xrload.tile([P, D], F32R, tag="xr", name=f"xr{c}_{sb}")
                s0 = (c * QPC + sb) * P
                eng.dma_start(xr, xb[s0 : s0 + P, :].bitcast(F32R))
                xr_tiles[(c, sb)] = xr

            # ---- PE startup: transpose weight/x pieces as they arrive,
            # with dummy matmuls sprinkled in to warm the HAM clock gate
            # (transpose-mode does not count as PE-busy for HAM).
            def emit_warm(n):
                for _ in range(n):
                    w = psO.tile([P, P], F32, tag="o", name="warm")
                    nc.tensor.matmul(w, ident_b, ident_b, start=True, stop=True)

            def emit_wtransS(stage, WT, o):
                # f32r-transpose one staged f32 weight piece [d-block o, e]
                # into WT[:, :, o-block]; the PSUM->SBUF copies cast bf16.
                for h in range(2):
                    ps_w = psT.tile([P, 3 * P], F32R, tag="t")
                    for eh in range(3):
                        eo = h * 3 + eh
                        nc.tensor.transpose(
                            ps_w[:, eh * P : (eh + 1) * P],
                            stage[:, 0, eo * P : (eo + 1) * P],
                            ident_r,
                        )
                    nc.vector.tensor_copy(
                        WT[:, h * 3 : h * 3 + 3, o * P : (o + 1) * P],
                        ps_w.bitcast(F32).rearrange("p (o s) -> p o s", o=3),
                    )

            def emit_xtrans0(sb):
                # bf16 PE transpose of a cast-loaded x(0) tile into XT
                xr = xf_tiles[sb]
                for h in range(2):
                    ps_t = psT.tile([P, 3 * P], BF16, tag="t")
                    for dh in range(3):
                        do = h * 3 + dh
                        nc.tensor.transpose(
                            ps_t[:, dh * P : (dh + 1) * P],
                            xr[:, do * P : (do + 1) * P],
                            ident_b,
                        )
                    nc.vector.tensor_copy(
                        XT[:, h * 3 : h * 3 + 3, sb * P : (sb + 1) * P],
                        ps_t.rearrange("p (o s) -> p o s", o=3),
                    )

            def emit_m_group(b2):
                # M column pair-group: Mt[:, :, b2*256:(b2+1)*256]
                for a in range(ND):
                    pm = psW.tile([P, 2 * P], F32, tag="w")
                    for eo in range(ND):
                        nc.tensor.matmul(
                            pm,
                            WqT[:, eo, a * P : (a + 1) * P],
                            WkT[:, eo, b2 * 2 * P : (b2 + 1) * 2 * P],
                            start=(eo == 0),
                            stop=(eo == ND - 1),
                        )
                    nc.vector.tensor_copy(
                        Mt[:, a, b2 * 2 * P : (b2 + 1) * 2 * P], pm
                    )

            # wq piece 0 from the two half-stages
            for h in range(2):
                ps_w = psT.tile([P, 3 * P], F32R, tag="t")
                for eh in range(3):
                    nc.tensor.transpose(
                        ps_w[:, eh * P : (eh + 1) * P],
                        wq0_half[h][:, 0, eh * P : (eh + 1) * P],
                        ident_r,
                    )
                nc.vector.tensor_copy(
                    WqT[:, h * 3 : h * 3 + 3, 0:P],
                    ps_w.bitcast(F32).rearrange("p (o s) -> p o s", o=3),
                )
                emit_warm(4)
            emit_wtransS(wq_stage[1], WqT, 1)
            emit_xtrans0(0)
            emit_warm(4)
            emit_wtransS(wq_stage[2], WqT, 2)
            emit_xtrans0(1)
            emit_warm(4)
            emit_wtransS(wq_stage[3], WqT, 3)
            emit_xtrans0(2)
            emit_warm(4)
            emit_wtransS(wq_stage[4], WqT, 4)
            emit_xtrans0(3)
            emit_wtransS(wq_stage[5], WqT, 5)
            emit_warm(4)

            def emit_xtrans(c):
                # PE transpose of chunk c into XT, grouped per o-block so
                # each DVE copy lands contiguously. Chunk 1 comes from the
                # bf16 cast-load; chunks 2-3 from f32 ring stages (f32r).
                for do in range(ND):
                    if c in xc_tiles:
                        xc = xc_tiles[c]
                        ps_t = psT.tile([P, QPC * P], BF16, tag="t")
                        for sb in range(QPC):
                            nc.tensor.transpose(
                                ps_t[:, sb * P : (sb + 1) * P],
                                xc[:, sb, do * P : (do + 1) * P],
                                ident_b,
                            )
                        nc.vector.tensor_copy(
                            XT[:, do, c * CH : (c + 1) * CH], ps_t
                        )
                    else:
                        ps_t = psT.tile([P, QPC * P], F32R, tag="t")
                        for sb in range(QPC):
                            nc.tensor.transpose(
                                ps_t[:, sb * P : (sb + 1) * P],
                                xr_tiles[(c, sb)][:, do * P : (do + 1) * P],
                                ident_r,
                            )
                        nc.vector.tensor_copy(
                            XT[:, do, c * CH : (c + 1) * CH], ps_t.bitcast(F32)
                        )

            def emit_vpiece(c, sb):
                xt_blk = XT[:, :, (c * QPC + sb) * P : (c * QPC + sb + 1) * P]
                pv0 = psW.tile([P, CH], F32, tag="w")
                for do in range(ND):
                    nc.tensor.matmul(
                        pv0,
                        xt_blk[:, do, :],
                        WV[:, do, 0:CH],
                        start=(do == 0),
                        stop=(do == ND - 1),
                    )
                cp0 = nc.scalar.copy if sb % 2 == 0 else nc.vector.tensor_copy
                cp0(V[:, c * QPC + sb, 0:CH], pv0)
                pv1 = psW.tile([P, CH], F32, tag="w")
                for do in range(ND):
                    nc.tensor.matmul(
                        pv1[:, 0 : D - CH],
                        xt_blk[:, do, :],
                        WV[:, do, CH:D],
                        start=(do == 0),
                        stop=(do == ND - 1),
                    )
                cp1 = nc.vector.tensor_copy if sb % 2 == 0 else nc.scalar.copy
                cp1(V[:, c * QPC + sb, CH:D], pv1[:, 0 : D - CH])

            def emit_bpiece(c, QT, eb):
                pq = psW.tile([P, CH], F32, tag="w")
                for do in range(ND):
                    nc.tensor.matmul(
                        pq,
                        Mt[:, do, eb * P : (eb + 1) * P],
                        XT[:, do, c * CH : (c + 1) * CH],
                        start=(do == 0),
                        stop=(do == ND - 1),
                    )
                nc.vector.tensor_copy(QT[:, eb, :], pq)

            # ---- interleaved startup tail: wk transposes and M groups as
            # wk pieces arrive; B(0) eb-pieces chase their M column group
            # (eb needs only group eb//2) and V(0) pieces (dep: WV + XT(0)
            # only) fill the remaining wk-DMA wait.
            QT0 = qtp.tile([P, ND, CH], BF16, tag="qt")
            emit_wtransS(wk_stage[0], WkT, 0)
            emit_wtransS(wk_stage[1], WkT, 1)
            emit_m_group(0)
            emit_bpiece(0, QT0, 0)
            emit_bpiece(0, QT0, 1)
            emit_wtransS(wk_stage[2], WkT, 2)
            emit_wtransS(wk_stage[3], WkT, 3)
            emit_m_group(1)
            emit_bpiece(0, QT0, 2)
            emit_bpiece(0, QT0, 3)
            emit_wtransS(wk_stage[4], WkT, 4)
            emit_wtransS(wk_stage[5], WkT, 5)
            emit_m_group(2)
            emit_bpiece(0, QT0, 4)
            emit_bpiece(0, QT0, 5)

            for c in range(NCH):
                # ---- projection: B = (x M)^T = Mt.T-contracted with XT
                # (chunk 0's B ran at startup, interleaved with M)
                if c == 0:
                    QT = QT0
                    for sb in range(QPC):
                        emit_xr_load(2, sb, nc.sync if sb % 2 == 0 else nc.scalar)
                else:
                    QT = qtp.tile([P, ND, CH], BF16, tag="qt")
                    for eb in range(ND):
                        emit_bpiece(c, QT, eb)
                        if c == 1 and eb < QPC:
                            emit_xr_load(3, eb, nc.sync if eb % 2 == 0 else nc.scalar)

                # ---- scores^T + exp; triangular mask on the diagonal corner
                for kb in range(QPC * (c + 1)):
                    i = kb - QPC * c
                    q0 = max(i, 0) * P
                    W = CH - q0
                    ps_s = psW.tile([P, CH], F32, tag="w")
                    for eo in range(ND):
                        nc.tensor.matmul(
                            ps_s[:, 0:W],
                            XT[:, eo, kb * P : (kb + 1) * P],
                            QT[:, eo, q0:CH],
                            start=(eo == 0),
                            stop=(eo == ND - 1),
                        )
                    nc.scalar.activation(PT[:, kb, q0:CH], ps_s[:, 0:W], EXP, scale=SCALE)
                    if i >= 0:
                        nc.vector.tensor_mul(
                            PT[:, kb, q0 : q0 + P], PT[:, kb, q0 : q0 + P], tri
                        )

                # ---- V projection for this chunk; next chunk's x
                # transposes run mid-V-phase
                for sb in range(QPC):
                    emit_vpiece(c, sb)
                    if sb == 2 and c + 1 < NCH:
                        emit_xtrans(c + 1)

                # ---- attn @ [V | 1], normalize, store
                for qs in range(QPC):
                    qb = c * QPC + qs
                    po1 = psW.tile([P, D + 2 - CH], F32, tag="w")
                    for kb in range(qb + 1):
                        nc.tensor.matmul(
                            po1,
                            PT[:, kb, qs * P : (qs + 1) * P],
                            V[:, kb, CH : D + 2],
                            start=(kb == 0),
                            stop=(kb == qb),
                        )
                    recip = rcp.tile([P, 1], F32, tag="rc")
                    nc.vector.reciprocal(recip, po1[:, D - CH : D - CH + 1])
                    o_sb = outp.tile([P, D], F32, tag="o")
                    nc.scalar.mul(o_sb[:, CH:D], po1[:, 0 : D - CH], recip)
                    nc.sync.dma_start(out_d[qb * P : (qb + 1) * P, CH:D], o_sb[:, CH:D])
                    if qb == NB - 1:
                        # last block: two 256-wide accumulation groups in
                        # separate PSUM tiles so the first half's mul+store
                        # hide under the second half's matmuls
                        H2 = CH // 2
                        for half in range(2):
                            lo, hi = half * H2, (half + 1) * H2
                            poh = psO.tile([P, H2], F32, tag="o")
                            for kb in range(qb + 1):
                                nc.tensor.matmul(
                                    poh,
                                    PT[:, kb, qs * P : (qs + 1) * P],
                                    V[:, kb, lo:hi],
                                    start=(kb == 0),
                                    stop=(kb == qb),
                                )
                            nc.scalar.mul(o_sb[:, lo:hi], poh, recip)
                            nc.sync.dma_start(
                                out_d[qb * P : (qb + 1) * P, lo:hi], o_sb[:, lo:hi]
                            )
                    else:
                        po0 = psO.tile([P, CH], F32, tag="o")
                        for kb in range(qb + 1):
                            nc.tensor.matmul(
                                po0,
                                PT[:, kb, qs * P : (qs + 1) * P],
                                V[:, kb, 0:CH],
                                start=(kb == 0),
                                stop=(kb == qb),
                            )
                        nc.scalar.mul(o_sb[:, 0:CH], po0, recip)
                        nc.sync.dma_start(
                            out_d[qb * P : (qb + 1) * P, 0:CH], o_sb[:, 0:CH]
                        )

    nc.finalize()
    return nc


_NC_CACHE = None


def _get_nc():
    global _NC_CACHE
    if _NC_CACHE is None:
        _NC_CACHE = _build_nc()
    return _NC_CACHE


def run(inputs, trace=False):
    x = np.asarray(inputs["x"], dtype=np.float32)
    wq = np.asarray(inputs["wq"], dtype=np.float32)
    wk = np.asarray(inputs["wk"], dtype=np.float32)
    wv = np.asarray(inputs["wv"], dtype=np.float32)
    nc = _get_nc()
    in_maps = [
        {"xb": np.ascontiguousarray(x[b]), "wq": wq, "wk": wk, "wv": wv}
        for b in range(B)
    ]
    res = run_bass_kernel_spmd(nc, in_maps, core_ids=list(range(B)), trace=trace)
    out = np.stack([r["out"] for r in res.results]).astype(np.float32)
    return out, res


def kernel(x, wq, wk, wv):
    out, _ = run({"x": x, "wq": wq, "wk": wk, "wv": wv}, trace=False)
    return out
